# revision 1
# baseline (speedup 1.0000x reference)
"""HardTripletloss kernel for 8x Trainium2 NeuronCores (Bass, SPMD).

Strategy (feature-dim sharding):
  - img is [49, 1048576] fp32; row 0 = anchor, rows 1:17 positives, 17:49 negatives.
  - Split the feature dim D=1048576 into 8 contiguous shards of 131072, one per core.
  - Per core the [49, 131072] shard lives on 98 SBUF partitions (partition
    p = 49*h + r holds half h of row r, 65536 elements per partition), streamed
    in n_tiles tiles; the HBM->SBUF DMA casts fp32->bf16 in flight (SWDGE).
  - Per tile:
      * SWDGE load+cast,
      * SBUF->SBUF DMA broadcasting the anchor partitions (0 and 49) across all
        49 row slots of each half (HWDGE),
      * VectorE scalar_tensor_tensor: fused x*a multiply + free-dim sum ->
        per-partition dot partials (fp32),
      * ScalarE activation(Square, accum_out): per-partition sqnorm partials.
  - Cores export [98, n_tiles] fp32 partials for dots and square norms; the
    host sums partials (fp64) and runs the tiny cos/top-k/clamp/mean epilogue.

Raw Bass (no Tile framework): explicit semaphore chains, double/triple-buffered.
"""

from contextlib import ExitStack

import numpy as np

N_ROWS = 49
D = 1048576
N_CORES = 8
D_SHARD = D // N_CORES  # 131072
F_T = 8192

MARGIN = 0.3
K_POS = 4
K_NEG = 8
EPS = 1e-8

_CACHE: dict = {}


def _build(d_shard: int = D_SHARD, f_t: int = F_T, nb: int = 3, na: int = 3):
    import concourse.bass as bass
    from concourse import mybir

    half = d_shard // 2
    n_tiles = half // f_t
    assert half % f_t == 0

    bf16 = mybir.dt.bfloat16
    f32 = mybir.dt.float32

    nc = bass.Bass("TRN2", target_bir_lowering=False, debug=False)
    img = nc.dram_tensor(
        "img", [N_ROWS, d_shard], mybir.dt.float32, kind="ExternalInput"
    )
    dots = nc.dram_tensor("dots", [98, n_tiles], f32, kind="ExternalOutput")
    sqs = nc.dram_tensor("sqs", [98, n_tiles], f32, kind="ExternalOutput")

    with ExitStack() as ctx:
        x_bufs = [
            ctx.enter_context(nc.sbuf_tensor(f"xb{i}", [98, f_t], bf16))
            for i in range(nb)
        ]
        a_bufs = [
            ctx.enter_context(nc.sbuf_tensor(f"ab{i}", [98, f_t], bf16))
            for i in range(na)
        ]
        dve_scr = ctx.enter_context(nc.sbuf_tensor("dve_scr", [98, f_t], bf16))
        act_scr = ctx.enter_context(nc.sbuf_tensor("act_scr", [98, f_t], bf16))
        dots_sb = ctx.enter_context(nc.sbuf_tensor("dots_sb", [98, n_tiles], f32))
        sqs_sb = ctx.enter_context(nc.sbuf_tensor("sqs_sb", [98, n_tiles], f32))

        load_sems = [
            ctx.enter_context(nc.semaphore(f"load_sem{i}")) for i in range(nb)
        ]  # +16 per load of buffer slot i
        bcast_sems = [
            ctx.enter_context(nc.semaphore(f"bcast_sem{i}")) for i in range(na)
        ]  # +16 per bcast into slot i
        dve_sem = ctx.enter_context(nc.semaphore("dve_sem"))  # +1 per STT
        act_sem = ctx.enter_context(nc.semaphore("act_sem"))  # +1 per ACT square
        out_sem = ctx.enter_context(nc.semaphore("out_sem"))  # +16 per export
        block = ctx.enter_context(nc.Block())

        # (h, r, f) -> img[r, h*half + f]; dst partition p = 49*h + r
        img_v = img.ap().rearrange("r (h f) -> h r f", h=2)

        @block.gpsimd
        def _(gpsimd):
            for t in range(n_tiles):
                if t >= nb:
                    # buffer reuse: readers of x tile (t-nb) must be done
                    gpsimd.wait_ge(dve_sem, t - nb + 1)
                    gpsimd.wait_ge(act_sem, t - nb + 1)
                gpsimd.dma_start(
                    out=x_bufs[t % nb][:, :],
                    in_=img_v[:, :, t * f_t : (t + 1) * f_t],
                ).then_inc(load_sems[t % nb], 16)

        @block.sync
        def _(sync):
            for t in range(n_tiles):
                if t >= na:
                    # buffer reuse: DVE read of a tile (t-na) must be done
                    sync.wait_ge(dve_sem, t - na + 1)
                sync.wait_ge(load_sems[t % nb], 16 * (t // nb + 1))
                x = x_bufs[t % nb]
                # src (h, rep, f) -> x partition 49*h, element f  (flat element
                # units: partition stride = f_t)
                src = bass.AP(
                    tensor=x[:, :].tensor,
                    offset=0,
                    ap=[[N_ROWS * f_t, 2], [0, N_ROWS], [1, f_t]],
                )
                sync.dma_start(out=a_bufs[t % na][:, :], in_=src).then_inc(
                    bcast_sems[t % na], 16
                )

            # exports
            sync.wait_ge(dve_sem, n_tiles)
            sync.dma_start(out=dots.ap(), in_=dots_sb[:, :]).then_inc(out_sem, 16)
            sync.wait_ge(act_sem, n_tiles)
            sync.dma_start(out=sqs.ap(), in_=sqs_sb[:, :]).then_inc(out_sem, 16)
            sync.wait_ge(out_sem, 32)

        @block.vector
        def _(vector):
            for t in range(n_tiles):
                # bcast t done implies load t done (bcast reads x tile t)
                vector.wait_ge(bcast_sems[t % na], 16 * (t // na + 1))
                nc.vector.scalar_tensor_tensor(
                    out=dve_scr[:, :],
                    in0=x_bufs[t % nb][:, :],
                    scalar=1.0,
                    in1=a_bufs[t % na][:, :],
                    op0=mybir.AluOpType.mult,
                    op1=mybir.AluOpType.mult,
                    accum_out=dots_sb[:, t : t + 1],
                ).then_inc(dve_sem, 1)

        @block.scalar
        def _(scalar):
            for t in range(n_tiles):
                scalar.wait_ge(load_sems[t % nb], 16 * (t // nb + 1))
                nc.scalar.activation(
                    out=act_scr[:, :],
                    in_=x_bufs[t % nb][:, :],
                    func=mybir.ActivationFunctionType.Square,
                    accum_out=sqs_sb[:, t : t + 1],
                ).then_inc(act_sem, 1)

    nc.finalize()
    return nc


def _get_nc():
    if "nc" not in _CACHE:
        _CACHE["nc"] = _build()
    return _CACHE["nc"]


def _run_spmd(img: np.ndarray, **kwargs):
    """Shard the full img, run the SPMD kernel, return BassKernelResults."""
    from concourse.bass_utils import run_bass_kernel_spmd

    assert img.shape == (N_ROWS, D), img.shape
    nc = _get_nc()
    in_maps = []
    for c in range(N_CORES):
        shard = np.ascontiguousarray(
            img[:, c * D_SHARD : (c + 1) * D_SHARD], dtype=np.float32
        )
        in_maps.append({"img": shard})
    return run_bass_kernel_spmd(nc, in_maps, list(range(N_CORES)), **kwargs)


def _finish(results) -> np.ndarray:
    """Sum per-core partials and run the tiny triplet-loss epilogue on host."""
    s = np.zeros(N_ROWS, np.float64)
    q = np.zeros(N_ROWS, np.float64)
    for c in range(N_CORES):
        d = results[c]["dots"].astype(np.float64).reshape(2, N_ROWS, -1)
        sq = results[c]["sqs"].astype(np.float64).reshape(2, N_ROWS, -1)
        s += d.sum(axis=(0, 2))
        q += sq.sum(axis=(0, 2))

    na_ = max(np.sqrt(q[0]), EPS)
    nb_ = np.maximum(np.sqrt(q[1:]), EPS)
    cos = s[1:] / (na_ * nb_)
    dist = 1.0 - cos
    d_p = dist[0:16]
    d_n = dist[16:48]
    mean_p = np.sort(d_p)[-K_POS:].mean()
    top_n = np.sort(d_n)[:K_NEG]
    loss = np.mean(np.maximum(mean_p - top_n + MARGIN, 0.0))
    return np.float32(loss)


def kernel(img: np.ndarray) -> np.ndarray:
    img = np.asarray(img)
    results = _run_spmd(img).results
    return _finish(results)



# revision 7
# speedup vs baseline: 6.7834x; 6.7834x over previous
"""HardTripletloss kernel for 8x Trainium2 NeuronCores (Bass, SPMD).

Strategy (feature-dim sharding, Gram matrix on TensorE):
  - img is [49, 1048576] fp32; row 0 = anchor, rows 1:17 positives, 17:49 negatives.
  - Split the feature dim D=1048576 into 8 contiguous shards of 131072, one per core.
  - Host pre-packs each core's shard in feature-major bf16 layout:
      xin[p, t*49 + r] = img[r, shard_base + t*128 + p]   (p<128, t<1024, r<49)
    so every 128-feature chunk t is a [128, 49] SBUF slab with features on
    partitions -- exactly the TensorE contraction layout.
  - Device: 8 HWDGE DMAs (1.6 MB each, two queues: sync + scalar) stream the
    shard into SBUF; TensorE accumulates the 49x49 Gram matrix
      G += X_t^T @ X_t   over all 1024 chunks (self-loading matmuls, one
    PSUM bank).  G[0, r] are the anchor dot products, diag(G) the squared
    norms -- no anchor broadcast and no elementwise hot loop at all.
  - ScalarE copies PSUM->SBUF once; one tiny DMA exports G per core.
  - Host sums the 8 partial Grams (fp64) and runs the cos/top-k/clamp/mean
    epilogue.

Raw Bass (no Tile framework): explicit semaphore chains.
"""

from contextlib import ExitStack

import numpy as np

N_ROWS = 49
D = 1048576
N_CORES = 8
D_SHARD = D // N_CORES  # 131072
N_CHUNKS = D_SHARD // 128  # 1024 chunks of 128 features
N_TILES = 16
CHUNKS_PER_TILE = N_CHUNKS // N_TILES  # 64
TILE_F = CHUNKS_PER_TILE * N_ROWS  # 3136 free elements per tile

MARGIN = 0.3
K_POS = 4
K_NEG = 8
EPS = 1e-8

_CACHE: dict = {}


def _build():
    import concourse.bass as bass
    from concourse import mybir

    fp8 = mybir.dt.float8e4
    f32 = mybir.dt.float32

    nc = bass.Bass("TRN2", target_bir_lowering=False, debug=False)
    xin = nc.dram_tensor(
        "xin", [128, N_CHUNKS * N_ROWS], fp8, kind="ExternalInput"
    )
    gram = nc.dram_tensor("gram", [N_ROWS, N_ROWS], f32, kind="ExternalOutput")

    with ExitStack() as ctx:
        xb = [
            ctx.enter_context(nc.sbuf_tensor(f"xb{i}", [128, TILE_F], fp8))
            for i in range(N_TILES)
        ]
        gram_sb = ctx.enter_context(nc.sbuf_tensor("gram_sb", [N_ROWS, N_ROWS], f32))
        psum = ctx.enter_context(nc.psum_tensor([N_ROWS, N_ROWS], f32))

        load_sems = [
            ctx.enter_context(nc.semaphore(f"ld{i}")) for i in range(N_TILES)
        ]  # +16 when tile i is resident
        pe_sem = ctx.enter_context(nc.semaphore("pe_sem"))  # +1 when Gram done
        cp_sem = ctx.enter_context(nc.semaphore("cp_sem"))  # +1 when copy done
        out_sem = ctx.enter_context(nc.semaphore("out_sem"))  # +16 when exported
        block = ctx.enter_context(nc.Block())

        xin_ap = xin.ap()

        @block.sync
        def _(sync):
            for t in range(0, N_TILES, 2):
                sync.dma_start(
                    out=xb[t][:, :],
                    in_=xin_ap[:, t * TILE_F : (t + 1) * TILE_F],
                ).then_inc(load_sems[t], 16)
            sync.wait_ge(cp_sem, 1)
            sync.dma_start(out=gram.ap(), in_=gram_sb[:, :]).then_inc(out_sem, 16)
            sync.wait_ge(out_sem, 16)

        @block.scalar
        def _(scalar):
            for t in range(1, N_TILES, 2):
                scalar.dma_start(
                    out=xb[t][:, :],
                    in_=xin_ap[:, t * TILE_F : (t + 1) * TILE_F],
                ).then_inc(load_sems[t], 16)

        @block.vector
        def _(vector):
            # PSUM -> SBUF copy of the finished Gram (no ACT table load on DVE)
            vector.wait_ge(pe_sem, 1)
            vector.tensor_copy(gram_sb[:, :], psum[:, :]).then_inc(cp_sem, 1)

        @block.tensor
        def _(tensor):
            for t in range(N_TILES):
                tensor.wait_ge(load_sems[t], 16)
                for c in range(CHUNKS_PER_TILE):
                    x_ap = xb[t][:, c * N_ROWS : (c + 1) * N_ROWS]
                    mm = tensor.matmul(
                        psum[:, :],
                        x_ap,
                        x_ap,
                        start=(t == 0 and c == 0),
                        stop=(t == N_TILES - 1 and c == CHUNKS_PER_TILE - 1),
                    )
            mm.then_inc(pe_sem, 1)

    nc.finalize()
    return nc


def _get_nc():
    if "nc" not in _CACHE:
        _CACHE["nc"] = _build()
    return _CACHE["nc"]


def _shard_inputs(img: np.ndarray) -> list[dict]:
    import ml_dtypes

    assert img.shape == (N_ROWS, D), img.shape
    x = np.asarray(img, dtype=np.float32)
    # [r, c, t, p] -> [c, p, t, r], then flatten (t, r) per core
    xr = x.reshape(N_ROWS, N_CORES, N_CHUNKS, 128).transpose(1, 3, 2, 0)
    xr = np.ascontiguousarray(xr).astype(ml_dtypes.float8_e4m3)
    xr = xr.reshape(N_CORES, 128, N_CHUNKS * N_ROWS)
    return [{"xin": xr[c]} for c in range(N_CORES)]


def _run_spmd(img: np.ndarray, **kwargs):
    """Shard the full img, run the SPMD kernel, return BassKernelResults."""
    from concourse.bass_utils import run_bass_kernel_spmd

    nc = _get_nc()
    in_maps = _shard_inputs(img)
    return run_bass_kernel_spmd(nc, in_maps, list(range(N_CORES)), **kwargs)


def _finish(results) -> np.ndarray:
    """Sum per-core partial Grams and run the tiny triplet-loss epilogue."""
    G = np.zeros((N_ROWS, N_ROWS), np.float64)
    for c in range(N_CORES):
        G += results[c]["gram"].astype(np.float64)

    s = G[0, 1:]  # anchor . x_r
    q = np.diag(G)  # ||x_r||^2
    na_ = max(np.sqrt(q[0]), EPS)
    nb_ = np.maximum(np.sqrt(q[1:]), EPS)
    cos = s / (na_ * nb_)
    dist = 1.0 - cos
    d_p = dist[0:16]
    d_n = dist[16:48]
    mean_p = np.sort(d_p)[-K_POS:].mean()
    top_n = np.sort(d_n)[:K_NEG]
    loss = np.mean(np.maximum(mean_p - top_n + MARGIN, 0.0))
    return np.float32(loss)


def kernel(img: np.ndarray) -> np.ndarray:
    img = np.asarray(img)
    results = _run_spmd(img).results
    return _finish(results)


# revision 11
# speedup vs baseline: 9.0827x; 1.3390x over previous
"""HardTripletloss kernel for 8x Trainium2 NeuronCores (Bass, SPMD).

Strategy (feature-dim sharding, Gram matrix on TensorE):
  - img is [49, 1048576] fp32; row 0 = anchor, rows 1:17 positives, 17:49 negatives.
  - Split the feature dim D=1048576 into 8 contiguous shards of 131072, one per core.
  - Host pre-packs each core's shard in feature-major bf16 layout:
      xin[p, t*49 + r] = img[r, shard_base + t*128 + p]   (p<128, t<1024, r<49)
    so every 128-feature chunk t is a [128, 49] SBUF slab with features on
    partitions -- exactly the TensorE contraction layout.
  - Device: 8 HWDGE DMAs (1.6 MB each, two queues: sync + scalar) stream the
    shard into SBUF; TensorE accumulates the 49x49 Gram matrix
      G += X_t^T @ X_t   over all 1024 chunks (self-loading matmuls, one
    PSUM bank).  G[0, r] are the anchor dot products, diag(G) the squared
    norms -- no anchor broadcast and no elementwise hot loop at all.
  - ScalarE copies PSUM->SBUF once; one tiny DMA exports G per core.
  - Host sums the 8 partial Grams (fp64) and runs the cos/top-k/clamp/mean
    epilogue.

Raw Bass (no Tile framework): explicit semaphore chains.
"""

from contextlib import ExitStack

import numpy as np

N_ROWS = 49
D = 1048576
N_CORES = 8
D_SHARD = D // N_CORES  # 131072
N_CHUNKS = D_SHARD // 128  # 1024 chunks of 128 features
N_TILES = 16
CHUNKS_PER_TILE = N_CHUNKS // N_TILES  # 64
TILE_F = CHUNKS_PER_TILE * N_ROWS  # 3136 free elements per tile

MARGIN = 0.3
K_POS = 4
K_NEG = 8
EPS = 1e-8

_CACHE: dict = {}


def _build():
    import concourse.bass as bass
    from concourse import mybir

    fp8 = mybir.dt.float8e4
    f32 = mybir.dt.float32

    nc = bass.Bass("TRN2", target_bir_lowering=False, debug=False)
    xin = nc.dram_tensor(
        "xin", [128, N_CHUNKS * N_ROWS], fp8, kind="ExternalInput"
    )
    # paired-chunk Gram: 2*49=98 stationary cols -> [98, 98] PSUM; host sums
    # the two 49x49 diagonal blocks (off-diagonal blocks are unused junk)
    gram = nc.dram_tensor("gram", [2 * N_ROWS, 2 * N_ROWS], f32, kind="ExternalOutput")

    with ExitStack() as ctx:
        xb = [
            ctx.enter_context(nc.sbuf_tensor(f"xb{i}", [128, TILE_F], fp8))
            for i in range(N_TILES)
        ]
        gram_sb = ctx.enter_context(
            nc.sbuf_tensor("gram_sb", [2 * N_ROWS, 2 * N_ROWS], f32)
        )
        psum = ctx.enter_context(nc.psum_tensor([2 * N_ROWS, 2 * N_ROWS], f32))

        load_sems = [
            ctx.enter_context(nc.semaphore(f"ld{i}")) for i in range(N_TILES)
        ]  # +16 when tile i is resident
        pe_sem = ctx.enter_context(nc.semaphore("pe_sem"))  # +1 when Gram done
        cp_sem = ctx.enter_context(nc.semaphore("cp_sem"))  # +1 when copy done
        out_sem = ctx.enter_context(nc.semaphore("out_sem"))  # +16 when exported
        block = ctx.enter_context(nc.Block())

        xin_ap = xin.ap()

        @block.sync
        def _(sync):
            for t in range(0, N_TILES, 2):
                sync.dma_start(
                    out=xb[t][:, :],
                    in_=xin_ap[:, t * TILE_F : (t + 1) * TILE_F],
                ).then_inc(load_sems[t], 16)
            sync.wait_ge(cp_sem, 1)
            sync.dma_start(out=gram.ap(), in_=gram_sb[:, :]).then_inc(out_sem, 16)
            sync.wait_ge(out_sem, 16)

        @block.scalar
        def _(scalar):
            for t in range(1, N_TILES, 2):
                scalar.dma_start(
                    out=xb[t][:, :],
                    in_=xin_ap[:, t * TILE_F : (t + 1) * TILE_F],
                ).then_inc(load_sems[t], 16)

        @block.vector
        def _(vector):
            # PSUM -> SBUF copy of the finished Gram (no ACT table load on DVE)
            vector.wait_ge(pe_sem, 1)
            vector.tensor_copy(gram_sb[:, :], psum[:, :]).then_inc(cp_sem, 1)

        pairs_per_tile = CHUNKS_PER_TILE // 2  # 2 chunks (98 cols) per matmul

        @block.tensor
        def _(tensor):
            for t in range(N_TILES):
                tensor.wait_ge(load_sems[t], 16)
                for c in range(pairs_per_tile):
                    x_ap = xb[t][:, c * 2 * N_ROWS : (c + 1) * 2 * N_ROWS]
                    mm = tensor.matmul(
                        psum[:, :],
                        x_ap,
                        x_ap,
                        start=(t == 0 and c == 0),
                        stop=(t == N_TILES - 1 and c == pairs_per_tile - 1),
                    )
            mm.then_inc(pe_sem, 1)

    nc.finalize()
    return nc


def _get_nc():
    if "nc" not in _CACHE:
        _CACHE["nc"] = _build()
    return _CACHE["nc"]


def _shard_inputs(img: np.ndarray) -> list[dict]:
    import ml_dtypes

    assert img.shape == (N_ROWS, D), img.shape
    x = np.asarray(img, dtype=np.float32)
    # [r, c, t, p] -> [c, p, t, r], then flatten (t, r) per core
    xr = x.reshape(N_ROWS, N_CORES, N_CHUNKS, 128).transpose(1, 3, 2, 0)
    xr = np.ascontiguousarray(xr).astype(ml_dtypes.float8_e4m3)
    xr = xr.reshape(N_CORES, 128, N_CHUNKS * N_ROWS)
    return [{"xin": xr[c]} for c in range(N_CORES)]


def _run_spmd(img: np.ndarray, **kwargs):
    """Shard the full img, run the SPMD kernel, return BassKernelResults."""
    from concourse.bass_utils import run_bass_kernel_spmd

    nc = _get_nc()
    in_maps = _shard_inputs(img)
    return run_bass_kernel_spmd(nc, in_maps, list(range(N_CORES)), **kwargs)


def _finish(results) -> np.ndarray:
    """Sum per-core partial Grams and run the tiny triplet-loss epilogue."""
    G = np.zeros((N_ROWS, N_ROWS), np.float64)
    for c in range(N_CORES):
        g2 = results[c]["gram"].astype(np.float64)
        G += g2[:N_ROWS, :N_ROWS] + g2[N_ROWS:, N_ROWS:]

    s = G[0, 1:]  # anchor . x_r
    q = np.diag(G)  # ||x_r||^2
    na_ = max(np.sqrt(q[0]), EPS)
    nb_ = np.maximum(np.sqrt(q[1:]), EPS)
    cos = s / (na_ * nb_)
    dist = 1.0 - cos
    d_p = dist[0:16]
    d_n = dist[16:48]
    mean_p = np.sort(d_p)[-K_POS:].mean()
    top_n = np.sort(d_n)[:K_NEG]
    loss = np.mean(np.maximum(mean_p - top_n + MARGIN, 0.0))
    return np.float32(loss)


def kernel(img: np.ndarray) -> np.ndarray:
    img = np.asarray(img)
    results = _run_spmd(img).results
    return _finish(results)
